# revision 17
# baseline (speedup 1.0000x reference)
"""Trainium2 Bass kernel for AngularFeaturePropagation (retrieval_knn).

Computation per batch element b (one NeuronCore per b, B=8 = n_cores):
  idx[n]  = argmin_m sqrt((lt[n]-ht[m])^2 + (lp[n]-hp[m])^2)      n<8192, m<2048
  interp  = high_feats[:, idx]                                     [128, 8192]
  cat     = [low_feats; interp]                                    [192, 8192]
  y0 = W0 @ cat  -> BN(global batch stats, over all cores) -> ReLU
  y1 = W1 @ h0   -> BN -> ReLU -> out                              [128, 8192]

v2 algorithm (windowed exact KNN):
  - Host sorts queries and candidates by theta per batch.  For each tile of
    128 consecutive sorted queries, an upper bound r_i on each query's NN
    distance (min over a seed window of ~320 candidates) gives a provably
    sufficient candidate window [lo, hi) in sorted-candidate index space
    (any candidate outside has |dtheta| > r_i >= NN dist).  Mean width ~480
    vs 2048 full scan -> ~4x less PE + DVE scan work, still EXACT.
  - SPMD: all 8 cores share one program.  Program slot widths = per-rank max
    of each core's sorted tile widths; each core maps its k-th widest tile
    to the slot with the k-th widest program width and pads its window to
    the program width with real neighboring candidates (superset = safe).
  - Scoring via fp32r hi/lo-split PE matmul (K=8 rows) as before, windowed.
    Pieces of <=1024 cols -> PSUM; DVE group-reduce(max, k=8) -> gmax;
    Max+MaxIndex top-1 group; exact fp32 rescore of the winning group's 8
    members (on GPSIMD) recovers the in-group offset; feature ap_gather.
  - Index band-replication (gpsimd gathers need idx copies in all 8 16-part
    bands, token-major) via ONE strided store + ONE stride-0 broadcast load
    instead of 8 strided DMAs (HWDGE fixed cost dominates small DMAs).
  - MLP in fp32r (11-bit mantissa, 1 PE cycle/row at >=256 cols): gathered
    interp feeds layer-0 directly with NO cast; layer-0 fused into the wave
    stream (PE/ACT run during scans); layer-1 h0 written as fp32r by ACT.
    BN stats: layer-0 ACT Copy/Square accum_out; layer-1 DVE bn_stats.
    AllReduce of (mean, E[x^2]); biases dropped (BN shift-invariant).
  - Output computed in sorted order; host un-permutes columns.
"""

import sys

if '/opt/trn_rl_repo' not in sys.path:
    sys.path.insert(0, '/opt/trn_rl_repo')

import numpy as np

import concourse.bass as bass
import concourse.bacc as bacc
import concourse.tile as tile
import concourse.mybir as mybir
from concourse import bass_utils, library_config

F32 = mybir.dt.float32
F32R = mybir.dt.float32r
BF16 = mybir.dt.bfloat16
U16 = mybir.dt.uint16
I16 = mybir.dt.int16
AF = mybir.ActivationFunctionType
OP = mybir.AluOpType
AX = mybir.AxisListType

B, N, M, C1, C2 = 8, 8192, 2048, 64, 128
NT = N // 128          # 64 query tiles / program slots
NG = M // 8            # 256 candidate groups of 8
NCH = N // 512         # 16 MLP chunks
NWV = 4                # waves of 16 slots
EPS = 1e-5
MINW = 256             # min slot width (fp32r needs >=256 cols for 1 cyc/row)

# which waves run the rescore on GPSIMD (else DVE)
POOL_RESCORE = (0, 1, 2, 3)


def _pieces(w):
    """Split a slot width into PSUM pieces: <=512 -> ps512, <=1024 -> ps1024."""
    out = []
    while w > 1024:
        out.append(1024)
        w -= 1024
    if w:
        out.append(w)
    return out


def build(PW, num_devices=8):
    PW = [int(w) for w in PW]
    assert len(PW) == NT and all(w % 8 == 0 and MINW <= w <= M for w in PW)
    TOTW = sum(PW)
    POFF = np.concatenate([[0], np.cumsum(PW)]).astype(int)   # slot offsets
    HOFF = [int(POFF[8 * h]) for h in range(8)]
    HW_W = [int(POFF[8 * (h + 1)] - POFF[8 * h]) for h in range(8)]
    PKMAX = max(HW_W)

    nc = bacc.Bacc("TRN2", target_bir_lowering=False, debug=False, num_devices=num_devices)

    # ---- per-core DRAM I/O ----
    d_qf = nc.dram_tensor("qf", [8, N], F32R, kind="ExternalInput")
    d_pack = nc.dram_tensor("pack", [8, TOTW], F32R, kind="ExternalInput")
    d_tbl = nc.dram_tensor("tbl", [NG, 64], F32, kind="ExternalInput")
    d_lt = nc.dram_tensor("ltt", [128, NT], F32, kind="ExternalInput")
    d_lp = nc.dram_tensor("lpt", [128, NT], F32, kind="ExternalInput")
    d_i8 = nc.dram_tensor("i8m16", [128, 8], F32, kind="ExternalInput")
    d_lf = nc.dram_tensor("lf", [C1, N], BF16, kind="ExternalInput")
    d_hf = nc.dram_tensor("hf", [C2, M], F32, kind="ExternalInput")
    d_w0lot = nc.dram_tensor("w0lot", [C1, 128], BF16, kind="ExternalInput")
    d_w0hit = nc.dram_tensor("w0hit", [C2, 128], F32, kind="ExternalInput")
    d_w1t = nc.dram_tensor("w1t", [128, 128], F32R, kind="ExternalInput")
    d_g0 = nc.dram_tensor("g0", [128, 1], F32, kind="ExternalInput")
    d_be0 = nc.dram_tensor("be0", [128, 1], F32, kind="ExternalInput")
    d_g1 = nc.dram_tensor("g1", [128, 1], F32, kind="ExternalInput")
    d_be1 = nc.dram_tensor("be1", [128, 1], F32, kind="ExternalInput")
    d_g0off = nc.dram_tensor("g0off", [128, NT], F32, kind="ExternalInput")
    d_g0u = nc.dram_tensor("g0u", [128, NT], U16, kind="ExternalInput")
    d_out = nc.dram_tensor("out", [128, N], F32, kind="ExternalOutput")

    with tile.TileContext(nc) as tc:
        with (
            tc.tile_pool(name="persist", bufs=1) as persist,
            tc.tile_pool(name="dram", bufs=1, space="DRAM") as dram,
            tc.tile_pool(name="big", bufs=1) as bigp,
            tc.tile_pool(name="ring", bufs=3) as ring,
            tc.tile_pool(name="h0ring", bufs=5) as h0ring,
            tc.tile_pool(name="recp", bufs=2) as recp,
            tc.tile_pool(name="qfp", bufs=2) as qfp,
            tc.tile_pool(name="pkp", bufs=2) as pkp,
            tc.tile_pool(name="gmaxp", bufs=6) as gmaxp,
            tc.tile_pool(name="sc8", bufs=8) as sc8,
            tc.tile_pool(name="wave", bufs=2) as wavep,
            tc.tile_pool(name="small", bufs=2) as small,
        ):
            # ---------------- persistent tiles ----------------
            hf_sb = persist.tile([C2, M], F32)
            nc.gpsimd.load_library(library_config.ap_gather)
            w0lot = persist.tile([C1, 128], BF16)
            w0hit = persist.tile([C2, 128], F32)
            w1t = persist.tile([128, 128], F32R)
            g0 = persist.tile([128, 1], F32)
            be0 = persist.tile([128, 1], F32)
            g1 = persist.tile([128, 1], F32)
            be1 = persist.tile([128, 1], F32)
            lf_sb = persist.tile([C1, N], BF16)
            lt_tok = persist.tile([128, NT], F32)
            lp_tok = persist.tile([128, NT], F32)
            i8m16 = persist.tile([128, 8], F32)
            g0off = persist.tile([128, NT], F32)
            g0u = persist.tile([128, NT], U16)

            def load_small_weights():
                nc.sync.dma_start(lt_tok[:], d_lt.ap())
                nc.sync.dma_start(lp_tok[:], d_lp.ap())
                nc.sync.dma_start(i8m16[:], d_i8.ap())
                nc.sync.dma_start(g0off[:], d_g0off.ap())
                nc.sync.dma_start(g0u[:], d_g0u.ap())
                nc.sync.dma_start(w0lot[:], d_w0lot.ap())
                nc.sync.dma_start(w0hit[:], d_w0hit.ap())
                nc.sync.dma_start(w1t[:], d_w1t.ap())
                nc.sync.dma_start(g0[:], d_g0.ap())
                nc.sync.dma_start(be0[:], d_be0.ap())
                nc.sync.dma_start(g1[:], d_g1.ap())
                nc.sync.dma_start(be1[:], d_be1.ap())

            gidx = persist.tile([128, NT * 8], U16)   # top-8 window-rel group ids
            gidxv = gidx[:].rearrange("p (t k) -> p t k", k=8)
            # token-major index roundtrip buffers (one per batch / wave so the
            # strided store's p-stride merges with the t-dim -> legal 3-dim DMA)
            d_gidT = [dram.tile([16, 64], U16, tag=f"gidT{i}", name=f"gidT{i}")
                      for i in range(8)]
            d_fiT = [dram.tile([16, 128], U16, tag=f"fiT{i}", name=f"fiT{i}")
                     for i in range(NWV)]
            g1w = persist.tile([128, N // 16], U16)
            fiw = persist.tile([128, N // 16], U16)
            st0sum = persist.tile([128, NCH], F32)
            st0sq = persist.tile([128, NCH], F32)
            st1 = persist.tile([128, NCH * 6], F32)

            interp = bigp.tile([C2, N], F32, tag="interp")        # 32KB
            y0 = bigp.tile([128, N], F32, tag="y0")               # 32KB
            interp3 = interp[:].rearrange("p (m d) -> p m d", d=1)
            hf3 = hf_sb[:].rearrange("p (m d) -> p m d", d=1)

            # ---------------- half-wave input streaming ----------------
            halfbufs = {}
            recbufs = {}

            def load_half(h):
                qfc = qfp.tile([8, 1024], F32R, tag="qfc", name="qfc")
                nc.sync.dma_start(qfc[:], d_qf.ap()[:, 1024 * h:1024 * (h + 1)])
                pk = pkp.tile([8, PKMAX], F32R, tag="pk", name="pk")
                nc.sync.dma_start(pk[:, 0:HW_W[h]],
                                  d_pack.ap()[:, HOFF[h]:HOFF[h] + HW_W[h]])
                halfbufs[h] = (qfc, pk)

            def emit_rescore(w, eng):
                ts = slice(16 * w, 16 * (w + 1))
                nt = 16
                rec1 = recbufs.pop(w)
                g1f = wavep.tile([128, 16], F32, tag="g1f", name="g1f")
                eng.tensor_copy(g1f[:], gidxv[:, ts, 0])
                eng.tensor_tensor(g1f[:], g1f[:], g0off[:, ts], op=OP.add)

                lt_b = lt_tok[:, ts].unsqueeze(2).broadcast_to((128, nt, 8))
                lp_b = lp_tok[:, ts].unsqueeze(2).broadcast_to((128, nt, 8))
                e = wavep.tile([128, 16, 8], F32, tag="e", name="e")
                dth = wavep.tile([128, 16, 8], F32, tag="dth", name="dth")
                dph = wavep.tile([128, 16, 8], F32, tag="dph", name="dph")
                eng.tensor_tensor(dth[:], lt_b, rec1[:, :, 0:8], op=OP.subtract)
                eng.tensor_tensor(dph[:], lp_b, rec1[:, :, 8:16], op=OP.subtract)
                eng.tensor_tensor(dth[:], dth[:], dth[:], op=OP.mult)
                eng.tensor_tensor(dph[:], dph[:], dph[:], op=OP.mult)
                eng.tensor_tensor(e[:], dth[:], dph[:], op=OP.add)

                emin = wavep.tile([128, 16], F32, tag="emin", name="emin")
                nc.vector.tensor_reduce(emin[:], e[:], axis=AX.X, op=OP.min)
                emin_b = emin[:].unsqueeze(2).broadcast_to((128, nt, 8))
                # sel = iota + (e - emin)*BIG: min over k = first index achieving emin
                sel = wavep.tile([128, 16, 8], F32, tag="sel", name="sel")
                i8_b = i8m16[:].unsqueeze(1).broadcast_to((128, nt, 8))
                eng.tensor_tensor(sel[:], e[:], emin_b, op=OP.subtract)
                eng.tensor_scalar(sel[:], sel[:], 1e20, None, op0=OP.mult)
                eng.tensor_tensor(sel[:], sel[:], i8_b, op=OP.add)
                off = wavep.tile([128, 16], F32, tag="off", name="off")
                nc.vector.tensor_reduce(off[:], sel[:], axis=AX.X, op=OP.min)

                # idx = (g1 + g0off)*8 + off
                fidx = wavep.tile([128, 16], F32, tag="fidx", name="fidx")
                eng.tensor_scalar(fidx[:], g1f[:], 8.0, None, op0=OP.mult)
                eng.tensor_tensor(fidx[:], fidx[:], off[:], op=OP.add)
                fidx_u = wavep.tile([128, 16], U16, tag="fidxu", name="fidxu")
                eng.tensor_copy(fidx_u[:], fidx[:])
                # token-major store + single broadcast band reload
                dst = d_fiT[w][:].rearrange("p (t j) -> j p t", j=8)
                nc.sync.dma_start(dst, fidx_u[:])
                src = d_fiT[w][:].unsqueeze(0).broadcast_to((8, 16, 128))
                nc.sync.dma_start(fiw[:, 128 * w:128 * (w + 1)], src)

            def emit_gather(w):
                nc.gpsimd.ap_gather(
                    interp3[:, 2048 * w:2048 * (w + 1), :], hf3,
                    fiw[:, 128 * w:128 * (w + 1)].bitcast(I16),
                    channels=128, num_elems=M, d=1, num_idxs=2048,
                )

            with (
                tc.tile_pool(name="ps512", bufs=3, space="PSUM") as ps512,
                tc.tile_pool(name="ps1024", bufs=2, space="PSUM") as ps1024,
                tc.tile_pool(name="l0ps", bufs=1, space="PSUM") as l0ps,
            ):
                def emit_l0(w, c):
                    cc = 4 * w + c
                    pw_ = l0ps.tile([128, 512], F32, tag="l0", name="l0pw")
                    nc.tensor.matmul(pw_[:], w0lot[:], lf_sb[:, 512 * cc:512 * (cc + 1)],
                                     start=True, stop=False)
                    nc.tensor.matmul(pw_[:], w0hit[:], interp[:, 512 * cc:512 * (cc + 1)],
                                     start=False, stop=True)
                    nc.scalar.activation(y0[:, 512 * cc:512 * (cc + 1)], pw_[:], AF.Copy,
                                         accum_out=st0sum[:, cc:cc + 1])
                    sq = ring.tile([128, 512], F32, tag="sq")
                    nc.scalar.activation(sq[:], pw_[:], AF.Square,
                                         accum_out=st0sq[:, cc:cc + 1])

                defer = {}
                load_half(0)
                load_half(1)
                defer.setdefault(2, []).append(load_small_weights)
                defer.setdefault(3, []).append(lambda: nc.sync.dma_start(hf_sb[:], d_hf.ap()))
                defer.setdefault(5, []).append(lambda: nc.sync.dma_start(lf_sb[:], d_lf.ap()))
                for t in range(NT + 20):
                    if t < NT:
                        w = t // 16
                        h, tt = divmod(t, 8)
                        qfc, pk = halfbufs[h]
                        if tt == 2 and h + 2 < 8:
                            load_half(h + 2)
                        soff = int(POFF[t]) - HOFF[h]
                        ngr = PW[t] // 8
                        gm = gmaxp.tile([128, 256], F32, tag="gm", name="gm")
                        col = 0
                        for psize in _pieces(PW[t]):
                            pool = ps512 if psize <= 512 else ps1024
                            ps = pool.tile([128, min(psize, 1024)], F32,
                                           tag=pool.name, name="ps")
                            for co in range(0, psize, 512):
                                cw = min(512, psize - co)
                                nc.tensor.matmul(
                                    ps[:, co:co + cw],
                                    qfc[:, 128 * tt:128 * (tt + 1)],
                                    pk[:, soff + col + co:soff + col + co + cw],
                                    start=True, stop=True,
                                )
                            nc.vector.tensor_reduce(
                                gm[:, col // 8:(col + psize) // 8],
                                ps[:, 0:psize].rearrange("p (g k) -> p g k", k=8),
                                axis=AX.X, op=OP.max)
                            col += psize
                        v8 = sc8.tile([128, 8], F32, tag="v8", name="v8")
                        nc.vector.max(v8[:], gm[:, 0:ngr])
                        nc.vector.max_index(gidx[:, 8 * t:8 * t + 8], v8[:], gm[:, 0:ngr])
                        if t % 8 == 7:
                            bi = t // 8
                            b0 = 8 * bi
                            if bi % 2 == 0:
                                recbufs[w] = recp.tile([128, 16, 64], F32,
                                                       tag="rec", name="rec")
                            gabs = sc8.tile([128, 8], U16, tag="gabs", name="gabs")
                            nc.vector.tensor_tensor(
                                gabs[:], gidxv[:, b0:b0 + 8, 0], g0u[:, b0:b0 + 8],
                                op=OP.add)
                            dst = d_gidT[bi][:].rearrange("p (t j) -> j p t", j=8)
                            nc.sync.dma_start(dst, gabs[:])
                            src = d_gidT[bi][:].unsqueeze(0).broadcast_to((8, 16, 64))
                            nc.sync.dma_start(g1w[:, 64 * bi:64 * (bi + 1)], src)
                            sw = 8 * (bi % 2)
                            nc.gpsimd.dma_gather(
                                recbufs[w][:, sw:sw + 8, :], d_tbl.ap(),
                                g1w[:, 64 * bi:64 * (bi + 1)].bitcast(I16),
                                num_idxs=1024, num_idxs_reg=1024, elem_size=64,
                            )
                        if t % 16 == 15:
                            eng = nc.gpsimd if w in POOL_RESCORE else nc.vector
                            defer.setdefault(t + 5, []).append(
                                lambda w_=w, e_=eng: emit_rescore(w_, e_))
                            defer.setdefault(t + 7, []).append(
                                lambda w_=w: emit_gather(w_))
                            for c in range(4):
                                defer.setdefault(t + 9 + c, []).append(
                                    lambda w_=w, c_=c: emit_l0(w_, c_))
                    for fn in defer.pop(t, []):
                        fn()
                assert not defer, sorted(defer)

            # ---------------- BN0 apply + layer 1 + BN1 + store ----------------
            d_ccin = dram.tile([128, 2], F32)
            d_ccout = dram.tile([128, 2], F32)
            d_ccin1 = dram.tile([128, 2], F32)
            d_ccout1 = dram.tile([128, 2], F32)

            def bn_scale_shift_agg(st, gam, bet, d_in, d_out_):
                """Reduce bn_stats records, AllReduce, return (scale, shift)."""
                ag = small.tile([128, 2], F32, tag="ag")
                nc.vector.bn_aggr(ag[:], st[:])
                msq = small.tile([128, 1], F32, tag="msq")
                nc.vector.tensor_mul(msq[:], ag[:, 0:1], ag[:, 0:1])
                cc = small.tile([128, 2], F32, tag="cca")
                nc.vector.tensor_copy(cc[:, 0:1], ag[:, 0:1])
                nc.vector.tensor_add(cc[:, 1:2], ag[:, 1:2], msq[:])
                return bn_finish(cc, gam, bet, d_in, d_out_)

            def bn_scale_shift(stsum, stsq, gam, bet, d_in, d_out_):
                """Reduce chunk sums, AllReduce (mean, E[x^2]), return (scale, shift)."""
                cc = small.tile([128, 2], F32, tag="cc")
                nc.vector.tensor_reduce(cc[:, 0:1], stsum[:], axis=AX.X, op=OP.add)
                nc.vector.tensor_reduce(cc[:, 1:2], stsq[:], axis=AX.X, op=OP.add)
                nc.vector.tensor_scalar_mul(cc[:], cc[:], 1.0 / N)
                return bn_finish(cc, gam, bet, d_in, d_out_)

            def bn_finish(cc, gam, bet, d_in, d_out_):
                nc.sync.dma_start(d_in[:], cc[:])
                if num_devices > 1:
                    nc.gpsimd.collective_compute(
                        "AllReduce", OP.add,
                        replica_groups=[list(range(num_devices))],
                        ins=[d_in[:].opt()], outs=[d_out_[:].opt()],
                    )
                else:
                    nc.sync.dma_start(d_out_[:], d_in[:])
                ccr = small.tile([128, 2], F32, tag="ccr")
                nc.sync.dma_start(ccr[:], d_out_[:])
                mu = small.tile([128, 1], F32, tag="mu")
                nc.vector.tensor_scalar_mul(mu[:], ccr[:, 0:1], 1.0 / num_devices)
                e2g = small.tile([128, 1], F32, tag="e2g")
                nc.vector.tensor_scalar_mul(e2g[:], ccr[:, 1:2], 1.0 / num_devices)
                musq = small.tile([128, 1], F32, tag="musq")
                nc.vector.tensor_mul(musq[:], mu[:], mu[:])
                var = small.tile([128, 1], F32, tag="var")
                nc.vector.tensor_sub(var[:], e2g[:], musq[:])
                vpe = small.tile([128, 1], F32, tag="vpe")
                nc.vector.tensor_scalar_add(vpe[:], var[:], EPS)
                sd = small.tile([128, 1], F32, tag="sd")
                nc.scalar.activation(sd[:], vpe[:], AF.Sqrt)
                rs = small.tile([128, 1], F32, tag="rs")
                nc.vector.reciprocal(rs[:], sd[:])
                sc_ = small.tile([128, 1], F32, tag="sc")
                nc.vector.tensor_mul(sc_[:], gam[:], rs[:])
                msc = small.tile([128, 1], F32, tag="msc")
                nc.vector.tensor_mul(msc[:], mu[:], sc_[:])
                sh = small.tile([128, 1], F32, tag="sh")
                nc.vector.tensor_sub(sh[:], bet[:], msc[:])
                return sc_, sh

            with tc.tile_pool(name="mpsum", bufs=4, space="PSUM") as mpsum:
                sc0, sh0 = bn_scale_shift(st0sum, st0sq, g0, be0, d_ccin, d_ccout)
                # reuse interp's 32KB buffer (interp fully consumed by layer 0)
                y1 = bigp.tile([128, N], F32, tag="interp")
                h0cs = {}
                for c in range(NCH // 2 + 4):
                    if c < NCH // 2:
                        h0c = h0ring.tile([128, 1024], F32R, tag="h0", name="h0c")
                        nc.scalar.activation(h0c[:],
                                             y0[:, 1024 * c:1024 * (c + 1)],
                                             AF.Relu, bias=sh0[:], scale=sc0[:])
                        h0cs[c] = h0c
                    if c >= 4:
                        cc_ = c - 4
                        h0c_ = h0cs.pop(cc_)
                        pw_ = mpsum.tile([128, 1024], F32, tag="mp", name="pw")
                        nc.tensor.matmul(pw_[:, 0:512], w1t[:], h0c_[:, 0:512],
                                         start=True, stop=True)
                        nc.tensor.matmul(pw_[:, 512:1024], w1t[:], h0c_[:, 512:1024],
                                         start=True, stop=True)
                        nc.vector.bn_stats(st1[:, 12 * cc_:12 * cc_ + 6], pw_[:, 0:512])
                        nc.vector.bn_stats(st1[:, 12 * cc_ + 6:12 * cc_ + 12],
                                           pw_[:, 512:1024])
                        nc.scalar.activation(y1[:, 1024 * cc_:1024 * (cc_ + 1)],
                                             pw_[:], AF.Copy)

                sc1, sh1 = bn_scale_shift_agg(st1, g1, be1, d_ccin1, d_ccout1)
                for c in range(8):
                    a_, b_ = 1024 * c, 1024 * (c + 1)
                    oc = ring.tile([128, 1024], F32, tag="osb", name="oc")
                    nc.scalar.activation(oc[:], y1[:, a_:b_],
                                         AF.Relu, bias=sh1[:], scale=sc1[:])
                    nc.sync.dma_start(d_out.ap()[:, a_:b_], oc[:])

    nc.compile()
    return nc


_NC_CACHE = {}


def _get_nc(pw_key):
    if pw_key not in _NC_CACHE:
        _NC_CACHE[pw_key] = build(list(pw_key))
    return _NC_CACHE[pw_key]


def _round_fp32r(x):
    """Round fp32 to fp32r (1+8+11 bits, RNE on bit 12)."""
    xi = np.asarray(x, np.float32).view(np.uint32).astype(np.uint64)
    lsb = (xi >> 12) & 1
    rounded = (xi + 0x7FF + lsb) & 0xFFFFF000
    return rounded.astype(np.uint32).view(np.float32)


def _split_fp32r(x):
    hi = _round_fp32r(x)
    lo = _round_fp32r(x.astype(np.float32) - hi)
    return hi, lo


# interleave order within a half-wave: widest, then median, alternating
_ILV = [0, 4, 1, 5, 2, 6, 3, 7]
# SLOT_RANK[s]: width-rank held by program slot s (ranks dealt over 8 halves)
SLOT_RANK = [0] * NT
for _hv in range(8):
    _rl = list(range(_hv, NT, 8))            # ranks dealt to this half
    for _i in range(8):
        SLOT_RANK[8 * _hv + _i] = _rl[_ILV[_i]]


def compute_schedule(inputs):
    lt = np.asarray(inputs['low_theta'], np.float32)
    lp = np.asarray(inputs['low_phi'], np.float32)
    ht = np.asarray(inputs['high_theta'], np.float32)
    hp = np.asarray(inputs['high_phi'], np.float32)

    qs_all, cs_all, lohi_all = [], [], []
    widths = np.zeros((B, NT), int)
    for b in range(B):
        qs = np.argsort(lt[b], kind='stable')
        cs = np.argsort(ht[b], kind='stable')
        qt, qp = lt[b][qs], lp[b][qs]
        ct, cp = ht[b][cs], hp[b][cs]
        lohi = []
        for t in range(NT):
            q0 = 128 * t
            c_lo = max(0, 32 * t - 96)
            c_hi = min(M, 32 * t + 128 + 96)
            d2 = ((qt[q0:q0 + 128, None] - ct[None, c_lo:c_hi]) ** 2
                  + (qp[q0:q0 + 128, None] - cp[None, c_lo:c_hi]) ** 2)
            r = np.sqrt(d2.min(axis=1))
            lo = int(np.searchsorted(ct, (qt[q0:q0 + 128] - r).min()))
            hi = int(np.searchsorted(ct, (qt[q0:q0 + 128] + r).max()))
            lo = (lo // 8) * 8
            hi = min(M, ((hi + 7) // 8) * 8)
            lohi.append((lo, hi))
            widths[b, t] = hi - lo
        qs_all.append(qs)
        cs_all.append(cs)
        lohi_all.append(lohi)

    # program widths: per-rank max over cores, clamped
    wsort = -np.sort(-widths, axis=1)
    pw_rank = np.maximum(wsort.max(axis=0), MINW)
    PW = [int(pw_rank[SLOT_RANK[s]]) for s in range(NT)]

    # per-core slot assignment: k-th widest tile -> slot with rank k
    tiles_of_slot = []
    los = np.zeros((B, NT), int)
    for b in range(B):
        rank_order = np.argsort(-widths[b], kind='stable')   # tile of rank k
        tos = [int(rank_order[SLOT_RANK[s]]) for s in range(NT)]
        tiles_of_slot.append(tos)
        for s in range(NT):
            lo, hi = lohi_all[b][tos[s]]
            w = PW[s]
            # center padding, keep [lo', lo'+w) within [0, M] and containing [lo, hi)
            lo2 = lo - (w - (hi - lo)) // 2
            lo2 = max(max(0, hi - w), min(lo2, min(lo, M - w)))
            lo2 = (lo2 // 8) * 8
            lo2 = max(max(0, hi - w), min(lo2, min(lo, M - w)))
            los[b, s] = lo2

    qperm = np.zeros((B, N), np.int64)
    for b in range(B):
        qs = qs_all[b]
        for s in range(NT):
            tsrc = tiles_of_slot[b][s]
            qperm[b, 128 * s:128 * (s + 1)] = qs[128 * tsrc:128 * (tsrc + 1)]

    return {
        'PW': tuple(PW), 'qperm': qperm, 'cs': cs_all, 'los': los,
    }


def make_in_maps(inputs, sched):
    lt = np.ascontiguousarray(inputs['low_theta'], np.float32)
    lp = np.ascontiguousarray(inputs['low_phi'], np.float32)
    lf = np.ascontiguousarray(inputs['low_feats'], np.float32)
    ht = np.ascontiguousarray(inputs['high_theta'], np.float32)
    hp = np.ascontiguousarray(inputs['high_phi'], np.float32)
    hf = np.ascontiguousarray(inputs['high_feats'], np.float32)
    W0 = np.asarray(inputs['W0'], np.float32)
    W1 = np.asarray(inputs['W1'], np.float32)
    import ml_dtypes
    bf = ml_dtypes.bfloat16
    w0lot = np.ascontiguousarray(W0[:, :C1].T).astype(bf)          # [64, 128]
    w0hit = np.ascontiguousarray(W0[:, C1:].T)                     # [128, 128]
    w1t = _round_fp32r(np.ascontiguousarray(W1.T))                 # [128, 128]
    g0 = np.ascontiguousarray(np.asarray(inputs['g0'], np.float32).reshape(128, 1))
    be0 = np.ascontiguousarray(np.asarray(inputs['beta0'], np.float32).reshape(128, 1))
    g1 = np.ascontiguousarray(np.asarray(inputs['g1'], np.float32).reshape(128, 1))
    be1 = np.ascontiguousarray(np.asarray(inputs['beta1'], np.float32).reshape(128, 1))
    i8m16 = np.ascontiguousarray(
        np.tile(np.arange(8, dtype=np.float32), (128, 1)))

    PW = sched['PW']
    POFF = np.concatenate([[0], np.cumsum(PW)]).astype(int)
    TOTW = int(POFF[-1])

    in_maps = []
    for b in range(B):
        qperm = sched['qperm'][b]
        cs = sched['cs'][b]
        los = sched['los'][b]
        qt, qp = lt[b][qperm], lp[b][qperm]          # slot-ordered queries
        ct, cp = ht[b][cs], hp[b][cs]                # sorted candidates
        q1h, q1l = _split_fp32r(2.0 * qt)
        q2h, q2l = _split_fp32r(2.0 * qp)
        c1h, c1l = _split_fp32r(ct)
        c2h, c2l = _split_fp32r(cp)
        w = -(ct.astype(np.float64) ** 2 + cp.astype(np.float64) ** 2)
        w = w.astype(np.float32)
        wh, wl = _split_fp32r(w)
        ones = np.ones(N, np.float32)
        # row pairing: s = q1h*c1h + q1h*c1l + q1l*c1h
        #            + q2h*c2h + q2h*c2l + q2l*c2h + 1*wh + 1*wl
        qf = np.stack([q1h, q1h, q1l, q2h, q2h, q2l, ones, ones], 0)
        cf = np.stack([c1h, c1l, c1h, c2h, c2l, c2h, wh, wl], 0)
        pack = np.zeros((8, TOTW), np.float32)
        for s in range(NT):
            lo = los[s]
            pack[:, POFF[s]:POFF[s + 1]] = cf[:, lo:lo + PW[s]]
        tbl = np.zeros((NG, 64), np.float32)
        tbl[:, 0:8] = ct.reshape(NG, 8)
        tbl[:, 8:16] = cp.reshape(NG, 8)
        g0off = np.ascontiguousarray(
            np.tile((los // 8).astype(np.float32), (128, 1)))
        g0u = np.ascontiguousarray(
            np.tile((los // 8).astype(np.uint16), (128, 1)))
        in_maps.append({
            "qf": np.ascontiguousarray(qf), "pack": np.ascontiguousarray(pack),
            "tbl": np.ascontiguousarray(tbl),
            "ltt": np.ascontiguousarray(qt.reshape(NT, 128).T),
            "lpt": np.ascontiguousarray(qp.reshape(NT, 128).T),
            "i8m16": i8m16,
            "lf": lf[b][:, qperm].astype(bf),
            "hf": np.ascontiguousarray(hf[b][:, cs]),
            "w0lot": w0lot, "w0hit": w0hit, "w1t": w1t,
            "g0": g0, "be0": be0, "g1": g1, "be1": be1,
            "g0off": g0off, "g0u": g0u,
        })
    return in_maps


def _prepare(inputs):
    sched = compute_schedule(inputs)
    nc = _get_nc(sched['PW'])
    in_maps = make_in_maps(inputs, sched)
    return nc, in_maps, sched


def kernel(**inputs):
    nc, in_maps, sched = _prepare(inputs)
    res = bass_utils.run_bass_kernel_spmd(nc, in_maps, core_ids=list(range(B)))
    out = np.empty((B, 128, N), np.float32)
    for b in range(B):
        out[b][:, sched['qperm'][b]] = res.results[b]["out"]
    return out
